# revision 17
# baseline (speedup 1.0000x reference)
"""Cumulative-probability head on 8 Trainium2 NeuronCores.

out[b, j] = sum_{i<=j} relu(x @ W_h^T + b_h)[b, i] + (x @ W_base^T + b_base)[b]

Data-parallel: x sharded along batch (1024 rows/core), weights replicated.

Per-core strategy (fp8 DoubleRow, v2):
  - x and W quantized host-side to fp8-e4m3 with power-of-2 scales Sx=16,
    Sw=512; hazard matmuls run DoubleRowSwInterleave (2 fp8 MACs/cell/cyc),
    256 contraction per chunk, 8 chunks.
  - Column split: each chunk does TWO matmuls, group A = N=258 covering
    hazard cols 0:256 plus the base column (col 256) riding in the same
    PSUM bank, group B = N=258 covering hazard cols 256:512 (+2 pad).
    This removes the per-chunk N=2 base matmuls entirely (their ~60ns
    dispatch floor cost ~3.8us of PE time in v1).
  - Bias is injected as a PSUM *opener*: a K=1 matmul (ones16 stationary x
    fp8 bias row) with start=True writes S*b into the bank before the
    chunk matmuls accumulate (start=False). Openers for wave-0 tiles run
    in the cold-clock DMA-fill window, so they are nearly free; no DVE
    bias adds and no closers. b_base also rides the opener (col 256 of
    group A), so the base activation needs no bias operand.
  - DMAs: host-side DRAM layouts are permuted so every transfer is
    per-partition contiguous: 4 w pair-DMAs ([128,2064B] rows), 4 wave-0
    x pair-DMAs ([128,2048B]), 4 wave-1 per-tile x DMAs ([128,2048B]),
    1 bias row. 13 input issues (~650ns each on the queues) vs 28 in v1.
  - Wave 0 (tiles 0-3): pair-outer, paced by the chunk-pair stream.
    Wave 1 (tiles 4-7): b-outer on resident data; t7 runs group A's 8
    chunks to a stop before group B so its scan overlaps B's matmuls.
  - Drain per tile: baset = Ident(psA[:,256])/S on ScalarE, Relu/S of
    psA[:,0:256] and psB[:,0:256] into bf16 haz, DVE tensor_tensor_scan
    (fp32 state) seeded with baset, bf16 out DMA on sync/scalar rings.
  - PSUM: 8 banks exactly = 4 tiles in flight x (psA, psB).
"""

import numpy as np
import ml_dtypes

import concourse.bass as bass
import concourse.tile as tile
from concourse import bacc, mybir
from concourse.bass_utils import run_bass_kernel_spmd

B, D, T = 8192, 2048, 512
NCORES = 8
BLOC = B // NCORES            # 1024 rows per core
NT = BLOC // 128              # 8 b-tiles per core
NCH = D // 256                # 8 contraction chunks (256 = 128 x 2 doublerow)
NG = 258                      # matmul N per column group (256 hazard + 2)
TP = 2 * NG                   # 516 = padded wt width per (chunk, slot)
SX = 16.0                     # x fp8 scale
SW = 512.0                    # W fp8 scale
S = SX * SW

F32 = mybir.dt.float32
BF16 = mybir.dt.bfloat16
F8 = mybir.dt.float8e4

F8NP = ml_dtypes.float8_e4m3
BF16NP = ml_dtypes.bfloat16

DR = mybir.MatmulPerfMode.DoubleRowSwInterleave
Relu = mybir.ActivationFunctionType.Relu
Ident = mybir.ActivationFunctionType.Identity


def _build_program():
    nc = bacc.Bacc("TRN2", target_bir_lowering=False, debug=False)

    # DRAM layouts (host-permuted for contiguous per-partition DMA rows):
    #  wt:  [128 p][8 c][2 slot][516 col]  (col = [h0:256|base|0 | h256:512|0|0])
    #  x0:  [4 pair][128 p][2 cc][4 bl][256 swi]   wave-0 rows 0:512
    #  x1:  [4 tile][128 p][8 c][256 swi]          wave-1 rows 512:1024
    #  b8:  [1][516]  = S/16 * [b_h[0:256], b_base, 0, b_h[256:512], 0, 0]
    wt_d = nc.dram_tensor("wt", [128, NCH, 2, TP], F8, kind="ExternalInput")
    x0_d = nc.dram_tensor("x0", [4, 128, 2048], F8, kind="ExternalInput")
    x1_d = nc.dram_tensor("x1", [4, 128, 2048], F8, kind="ExternalInput")
    b8_d = nc.dram_tensor("b8", [1, TP], F8, kind="ExternalInput")
    out_d = nc.dram_tensor("out", [BLOC, T], BF16, kind="ExternalOutput")

    with tile.TileContext(nc) as tc:
        with (
            tc.tile_pool(name="consts", bufs=1) as consts,
            tc.tile_pool(name="wt", bufs=1) as wtp,
            tc.tile_pool(name="xt", bufs=1) as xtp,
            tc.tile_pool(name="haz", bufs=4) as hazp,
            tc.tile_pool(name="outp", bufs=4) as outp,
            tc.tile_pool(name="ps", bufs=8, space="PSUM") as psp,
        ):
            # Small consts. zeros memset first on the DVE queue: the HAM
            # warm-up dummies read it, and they must be full-K (128
            # partitions) matmuls — K=1 matmuls don't register as PE
            # activity for the clock-gate monitor.
            zeros = consts.tile([128, T], BF16, tag="zeros")
            nc.vector.memset(zeros, 0.0)
            ones16 = consts.tile([1, 128], F8, tag="ones16")
            nc.gpsimd.memset(ones16, 16.0)
            b8t = consts.tile([1, TP], F8, tag="b8t")

            wt_tiles = [
                wtp.tile([128, 2, 2, TP], F8, tag=f"w{i}", name=f"w{i}")
                for i in range(4)
            ]
            x0_tiles = [
                xtp.tile([128, 2048], F8, tag=f"x0_{i}", name=f"x0_{i}")
                for i in range(4)
            ]
            x1_tiles = [
                xtp.tile([128, 2048], F8, tag=f"x1_{i}", name=f"x1_{i}")
                for i in range(4)
            ]

            # Input DMA issue order per ring (in-queue order = issue order).
            # Measured: rings spin up ~0.8/1.8/2.7us after their first issue,
            # in first-issue order, and share ~430GB/s. So the critical lead
            # pieces (bias row, w chunk 0, x0 chunk 0) all ride sync, and
            # chunks 0/1 are single-chunk DMAs for fine-grained pacing.
            # ~1.03MB per ring, strictly deadline-ordered: rings share
            # ~320-430GB/s once all three stream, and which ring spins up
            # first is run-variable — so chunk 0's pieces are split small
            # across ALL rings and every ring leads with its earliest-needed
            # bytes.
            sync_q = [
                (b8t, b8_d[0:1, :]),
                (wt_tiles[0][:, 0:1, 0:1], wt_d[:, 0:1, 0:1]),
                (x0_tiles[0][:, 1024:2048], x0_d[0, :, 1024:2048]),
                (x0_tiles[2], x0_d[2]),
                (x1_tiles[0], x1_d[0]),
                (x1_tiles[2], x1_d[2]),
            ]
            scalar_q = [
                (wt_tiles[0][:, 0:1, 1:2], wt_d[:, 0:1, 1:2]),
                (x0_tiles[0][:, 0:512], x0_d[0, :, 0:512]),
                (wt_tiles[0][:, 1:2], wt_d[:, 1:2]),
                (x0_tiles[1], x0_d[1]),
                (x0_tiles[3], x0_d[3]),
                (x1_tiles[1], x1_d[1]),
            ]
            gpsimd_q = [
                (x0_tiles[0][:, 512:1024], x0_d[0, :, 512:1024]),
                (wt_tiles[1], wt_d[:, 2:4]),
                (wt_tiles[2], wt_d[:, 4:6]),
                (wt_tiles[3], wt_d[:, 6:8]),
                (x1_tiles[3], x1_d[3]),
            ]
            for ring, q in (
                (nc.sync, sync_q),
                (nc.scalar, scalar_q),
                (nc.gpsimd, gpsimd_q),
            ):
                for dst, src in q:
                    ring.dma_start(out=dst, in_=src)

            def rhs_w(c, g):
                # [128, 2 slot, 258] slice of the pair tile for chunk c.
                return wt_tiles[c // 2][:, c % 2, :, NG * g : NG * (g + 1)]

            def lhsT_w0(pair, cc, bl):
                # SWI stationary: block q = 2*(127-m) + i, at 1024*cc+256*bl.
                sl = x0_tiles[pair][:, 1024 * cc + 256 * bl : 1024 * cc + 256 * (bl + 1)]
                return bass.AP(
                    tensor=sl.tensor,
                    offset=sl.offset,
                    ap=[list(sl.ap[0]), [1, 2], [2, 128]],
                )

            def lhsT_w1(t, c):
                sl = x1_tiles[t][:, 256 * c : 256 * (c + 1)]
                return bass.AP(
                    tensor=sl.tensor,
                    offset=sl.offset,
                    ap=[list(sl.ap[0]), [1, 2], [2, 128]],
                )

            def opener(ps, g):
                # ps[:, 0:258] = 16 * b8[g] = S*b  (start=True clears bank).
                nc.tensor.matmul(
                    ps[:, 0:NG],
                    ones16[0:1, :],
                    b8t[0:1, NG * g : NG * (g + 1)],
                    start=True,
                    stop=False,
                )

            def chunk_mm(ps, lhsT, c, g, stop):
                nc.tensor.matmul(
                    ps[:, 0:NG],
                    lhsT,
                    rhs_w(c, g),
                    start=False,
                    stop=stop,
                    perf_mode=DR,
                )

            # PSUM tiles: 8 banks; tile t uses (psA, psB); pool rotation
            # reuses wave-0 banks for wave-1 after their drains.
            def ps_pair(t):
                a = psp.tile([128, 512], F32, tag="ps", name=f"psA{t}")
                b = psp.tile([128, 512], F32, tag="ps", name=f"psB{t}")
                return a, b

            # Out rings: early tiles can ride the slow software ring, but the
            # final tiles must ride fast rings or their DMA drain extends the
            # epilogue. Scalar gets only t7's tail half (nothing queued after
            # its last ACT, so the in-order scan wait is harmless there).
            out_ring_of = {0: nc.gpsimd, 1: nc.sync, 2: nc.gpsimd, 3: nc.sync,
                           4: nc.gpsimd, 5: nc.sync, 6: nc.sync}
            t7_rings = [nc.sync, nc.scalar]

            def drain(t, psA, psB, split_dma=1):
                baset = hazp.tile([128, 1], BF16, tag="base", name=f"base{t}")
                nc.scalar.activation(
                    out=baset, in_=psA[:, 256:257], func=Ident, scale=1.0 / S
                )
                haz = hazp.tile([128, T], BF16, tag="haz", name=f"haz{t}")
                nc.scalar.activation(
                    out=haz[:, 0:256], in_=psA[:, 0:256], func=Relu, scale=1.0 / S
                )
                nc.scalar.activation(
                    out=haz[:, 256:512], in_=psB[:, 0:256], func=Relu, scale=1.0 / S
                )
                cum = outp.tile([128, T], BF16, tag="cum", name=f"cum{t}")
                H = T // split_dma
                for q in range(split_dma):
                    lo, hi = q * H, (q + 1) * H
                    nc.vector.tensor_tensor_scan(
                        out=cum[:, lo:hi],
                        data0=haz[:, lo:hi],
                        data1=zeros[:, lo:hi],
                        initial=baset if q == 0 else cum[:, lo - 1 : lo],
                        op0=mybir.AluOpType.add,
                        op1=mybir.AluOpType.add,
                    )
                    ring = t7_rings[q] if t == 7 else out_ring_of[t]
                    ring.dma_start(
                        out=out_d[128 * t : 128 * (t + 1), lo:hi], in_=cum[:, lo:hi]
                    )

            # ---- wave 0: tiles 0-3, pair-outer ----
            psw0 = [ps_pair(t) for t in range(4)]

            # HAM warm-up: 8 full-K bf16 dummies (~3.4us cold) so the clock
            # gate opens around the time the first chunk matmuls can start.
            for _ in range(8):
                nc.tensor.matmul(
                    psw0[0][0][:], zeros[:, 0:128], zeros[:, 0:T],
                    start=True, stop=True,
                )
            # Openers t0/t1, chunk-0 work for t0/t1, openers t2/t3, etc —
            # staggered so chunk mms start as soon as chunk 0 lands.
            for t in (0, 1):
                opener(psw0[t][0], 0)
                opener(psw0[t][1], 1)
            for t in (0, 1):
                lh = lhsT_w0(0, 0, t)
                chunk_mm(psw0[t][0], lh, 0, 0, False)
                chunk_mm(psw0[t][1], lh, 0, 1, False)
            for t in (2, 3):
                opener(psw0[t][0], 0)
                opener(psw0[t][1], 1)
            for t in (2, 3):
                lh = lhsT_w0(0, 0, t)
                chunk_mm(psw0[t][0], lh, 0, 0, False)
                chunk_mm(psw0[t][1], lh, 0, 1, False)
            for t in range(4):
                lh = lhsT_w0(0, 1, t)
                chunk_mm(psw0[t][0], lh, 1, 0, False)
                chunk_mm(psw0[t][1], lh, 1, 1, False)
            for pair in (1, 2, 3):
                for t in range(4):
                    for cc in (0, 1):
                        c = 2 * pair + cc
                        last = pair == 3 and cc == 1
                        lh = lhsT_w0(pair, cc, t)
                        chunk_mm(psw0[t][0], lh, c, 0, last)
                        chunk_mm(psw0[t][1], lh, c, 1, last)
            for t in range(4):
                drain(t, psw0[t][0], psw0[t][1])

            # ---- wave 1: tiles 4-7, b-outer ----
            for t in range(4, 8):
                psA, psB = ps_pair(t)
                if t < 7:
                    opener(psA, 0)
                    opener(psB, 1)
                    for c in range(NCH):
                        lh = lhsT_w1(t - 4, c)
                        chunk_mm(psA, lh, c, 0, c == NCH - 1)
                        chunk_mm(psB, lh, c, 1, c == NCH - 1)
                    drain(t, psA, psB)
                else:
                    # t7: finish group A first so its drain overlaps B's mms.
                    opener(psA, 0)
                    for c in range(NCH):
                        chunk_mm(psA, lhsT_w1(3, c), c, 0, c == NCH - 1)
                    opener(psB, 1)
                    for c in range(NCH):
                        chunk_mm(psB, lhsT_w1(3, c), c, 1, c == NCH - 1)
                    drain(t, psA, psB, split_dma=2)

    nc.compile()
    return nc


_NC_CACHE = None


def prep_in_maps(x, W_hazard, b_hazard, W_base, b_base):
    x = np.asarray(x, np.float32)
    Wh = np.asarray(W_hazard, np.float32)
    bh = np.asarray(b_hazard, np.float32)
    Wb = np.asarray(W_base, np.float32).reshape(D)
    bb = np.asarray(b_base, np.float32).reshape(1)

    # Column layout per (chunk, slot): [h0:256 | base | 0 | h256:512 | 0 | 0]
    wcols = np.zeros((D, TP), np.float32)
    wcols[:, 0:256] = Wh[0:256].T * SW
    wcols[:, 256] = Wb * SW
    wcols[:, NG : NG + 256] = Wh[256:512].T * SW
    np.clip(wcols, -240.0, 240.0, out=wcols)
    w8 = wcols.astype(F8NP)  # [2048 k, 516]
    # k = 256*c + 2*p + s  ->  [p][c][s][516]
    wt = np.ascontiguousarray(
        w8.reshape(NCH, 128, 2, TP).transpose(1, 0, 2, 3)
    )  # [128, 8, 2, 516]

    b8 = np.zeros((1, TP), np.float32)
    b8[0, 0:256] = bh[0:256] * (S / 16.0)
    b8[0, 256] = bb[0] * (S / 16.0)
    b8[0, NG : NG + 256] = bh[256:512] * (S / 16.0)
    np.clip(b8, -240.0, 240.0, out=b8)
    b8 = b8.astype(F8NP)

    x8 = np.clip(x * SX, -240.0, 240.0).astype(F8NP)  # [B, D]
    in_maps = []
    for i in range(NCORES):
        xs = x8[BLOC * i : BLOC * (i + 1)]  # [1024, D]
        # [tile 8, m 128, c 8, p 128, i 2], m reversed for SWI blocks.
        Y = xs.reshape(NT, 128, NCH, 128, 2)[:, ::-1, :, :, :]
        # wave 0: [pair][p][cc][bl][ (127-m, i) ] -> [4, 128, 2048]
        Y0 = Y[0:4]  # [bl 4, m_r, c, p, i]
        x0 = np.ascontiguousarray(
            Y0.transpose(2, 3, 0, 1, 4)  # [c, p, bl, m_r, i]
            .reshape(4, 2, 128, 4, 128, 2)  # [pair, cc, p, bl, m_r, i]
            .transpose(0, 2, 1, 3, 4, 5)  # [pair, p, cc, bl, m_r, i]
            .reshape(4, 128, 2048)
        )
        # wave 1: [tile][p][c][ (127-m, i) ] -> [4, 128, 2048]
        Y1 = Y[4:8]  # [t, m_r, c, p, i]
        x1 = np.ascontiguousarray(
            Y1.transpose(0, 3, 2, 1, 4).reshape(4, 128, 2048)
        )
        in_maps.append({"x0": x0, "x1": x1, "wt": wt, "b8": b8})
    return in_maps


def kernel(x, W_hazard, b_hazard, W_base, b_base):
    global _NC_CACHE
    if _NC_CACHE is None:
        _NC_CACHE = _build_program()
    in_maps = prep_in_maps(x, W_hazard, b_hazard, W_base, b_base)
    res = run_bass_kernel_spmd(_NC_CACHE, in_maps, list(range(NCORES)))
    return np.concatenate(
        [res.results[i]["out"].astype(np.float32) for i in range(NCORES)], axis=0
    )
